# revision 25
# baseline (speedup 1.0000x reference)
"""Multi-head self-attention with RoPE on 8 Trainium2 NeuronCores.

Sharding: core c = (b, g) with b = c // 4 (batch of 2), g = c % 4 (head
group of 4 heads out of 16). Each core computes Q/K/V projections for its
4 heads on its batch, RoPE, causal attention, producing a context slab
(256 features x 2048 tokens). A 4-rank AllGather per batch group
assembles the full (1024, 2048) context; each core then applies a
256-column slice of the output projection.

v2 notes vs the original staged kernel:
- bf16 datapath everywhere off-PSUM (x, weights, rope tables, q/k, probs,
  v, context, collective payload); PSUM accumulation stays fp32.
- Causal masking via a precomputed (128,128) lower-triangle bf16 tile
  multiplied into the probs window on the Vector engine; the gpsimd
  affine_select (which shared the Pool queue with the collectives and
  head-of-line blocked the pipeline) is gone.
- Diagonal key-chunks only compute/exp/PV the query window that is
  actually unmasked (128*i .. 512).
- No PE keep-warm filler matmuls.
- DMAs that depend on collective completion (AllGather output reload) are
  triggered from the GpSimd queue; rope block-swap SBUF copies from the
  Activation queue; so the Sync queue never head-of-line blocks on a
  pending collective.

Self-contained: hardcodes all shapes; builds and compiles the SPMD Bass
program once per process.
"""
import os
import numpy as np

import concourse.bass as bass
import concourse.mybir as mybir
import concourse.tile as tile
from concourse import bacc
from concourse.bass_utils import run_bass_kernel_spmd

B, S, D, H, DK = 2, 2048, 1024, 16, 64
NF = DK // 2            # 32 rotary frequencies
HPC = 4                 # heads per core
GF = HPC * DK           # 256 features per core
NCORES = 8
THETA = 10000.0
F32 = mybir.dt.float32
BF16 = mybir.dt.bfloat16
AF = mybir.ActivationFunctionType

_CACHE: dict = {}


def _emit(nc: bacc.Bacc, debug: bool = False):
    xT = nc.dram_tensor("xT", [D, S], BF16, kind="ExternalInput").ap()
    wqT = nc.dram_tensor("wqT", [D, GF], BF16, kind="ExternalInput").ap()
    wkT = nc.dram_tensor("wkT", [D, GF], BF16, kind="ExternalInput").ap()
    wvT = nc.dram_tensor("wvT", [D, GF], BF16, kind="ExternalInput").ap()
    woT = nc.dram_tensor("woT", [D, GF], BF16, kind="ExternalInput").ap()
    cs_d = nc.dram_tensor("cs", [128, S], BF16, kind="ExternalInput").ap()
    ss_d = nc.dram_tensor("ss", [128, S], BF16, kind="ExternalInput").ap()
    ones_d = nc.dram_tensor("ones", [128, 64], BF16, kind="ExternalInput").ap()
    tri_d = nc.dram_tensor("tri", [128, 128], BF16, kind="ExternalInput").ap()
    out_d = nc.dram_tensor("out", [GF, S], F32, kind="ExternalOutput").ap()
    dbg = {}
    if debug:
        for nm in ("dbg_qt0", "dbg_qt1", "dbg_kt0", "dbg_kt1"):
            dbg[nm] = nc.dram_tensor(nm, [128, S], BF16, kind="ExternalOutput").ap()
        dbg["dbg_v"] = nc.dram_tensor("dbg_v", [128, (S // 128) * 260], BF16,
                                      kind="ExternalOutput").ap()
        for p in range(2):
            dbg[f"dbg_ctx{p}"] = nc.dram_tensor(f"dbg_ctx{p}", [128, S], BF16,
                                                kind="ExternalOutput").ap()
        dbg["dbg_den"] = nc.dram_tensor("dbg_den", [8, 1024], F32,
                                        kind="ExternalOutput").ap()
        dbg["dbg_rcp"] = nc.dram_tensor("dbg_rcp", [8, 1024], BF16,
                                        kind="ExternalOutput").ap()
        dbg["dbg_pbc"] = nc.dram_tensor("dbg_pbc", [8, 1024], F32,
                                        kind="ExternalOutput").ap()

    NKT = D // 128       # 8 contraction tiles for projections
    NJ = S // 512        # 4 token 512-blocks
    NQB = S // 512       # 4 query 512-blocks

    with tile.TileContext(nc) as tc:
        with (
            tc.tile_pool(name="singles", bufs=1) as singles,
            tc.tile_pool(name="dram", bufs=1, space="DRAM") as dram,
        ):
            # ---- resident tiles ----
            wq_sb = singles.tile([128, NKT, GF], BF16, tag="wq")
            wk_sb = singles.tile([128, NKT, GF], BF16, tag="wk")
            wv_sb = singles.tile([128, NKT, GF], BF16, tag="wv")
            nc.sync.dma_start(out=wq_sb[:], in_=wqT.rearrange("(k p) n -> p k n", p=128))
            nc.sync.dma_start(out=wk_sb[:], in_=wkT.rearrange("(k p) n -> p k n", p=128))
            nc.sync.dma_start(out=wv_sb[:], in_=wvT.rearrange("(k p) n -> p k n", p=128))
            cs_sb = singles.tile([128, S], BF16, tag="cs")
            ss_sb = singles.tile([128, S], BF16, tag="ss")
            nc.sync.dma_start(out=cs_sb[:], in_=cs_d[:])
            nc.sync.dma_start(out=ss_sb[:], in_=ss_d[:])
            ones_sb = singles.tile([128, 64], BF16, tag="ones")
            nc.sync.dma_start(out=ones_sb[:], in_=ones_d[:])
            tri_sb = singles.tile([128, 128], BF16, tag="tri")
            nc.sync.dma_start(out=tri_sb[:], in_=tri_d[:])
            wo_sb = singles.tile([128, NKT, GF], BF16, tag="wo")
            nc.sync.dma_start(out=wo_sb[:], in_=woT.rearrange("(k p) n -> p k n", p=128))

            # roped Q^T / K^T: 2 tiles each, rows = [headA(64) | headB(64)],
            # within each head block [x0(32) | x1(32)]
            qt = [singles.tile([128, S], BF16, tag=f"qt{m}", name=f"qt{m}") for m in range(2)]
            kt = [singles.tile([128, S], BF16, tag=f"kt{m}", name=f"kt{m}") for m in range(2)]
            # V with per-head ones column: head h occupies cols 65h..65h+63,
            # col 65h+64 is 1.0 (softmax denominator rides the PV matmul)
            NVT = S // 128
            v_sb = singles.tile([128, NVT, 4 * 65], BF16, tag="v")
            nc.vector.tensor_copy(
                v_sb.rearrange("p t (h e) -> p t h e", h=4)[:, :, :, 64:65],
                ones_sb.rearrange("p (t h) -> p t h", t=NVT)[:, :, :, None])
            # context per p-pair: rows [headA(64) | headB(64)]
            ctx2_sb = [singles.tile([128, S], BF16, tag=f"ctx{p}", name=f"ctx{p}")
                       for p in range(2)]

            # ---- phase 1: QKV projections + RoPE ----
            with (
                tc.tile_pool(name="xin", bufs=16) as xin,
                tc.tile_pool(name="qkraw", bufs=4) as qkraw,
                tc.tile_pool(name="ropetmp", bufs=4) as ropetmp,
                tc.tile_pool(name="ps_qk", bufs=2, space="PSUM") as ps_qk,
                tc.tile_pool(name="ps_v", bufs=2, space="PSUM") as ps_v,
            ):
                for j in range(NJ):
                    csl = slice(512 * j, 512 * (j + 1))
                    xts = []
                    for k in range(NKT):
                        xt_ = xin.tile([128, 512], BF16)
                        nc.sync.dma_start(out=xt_[:], in_=xT[128 * k:128 * (k + 1), csl])
                        xts.append(xt_)
                    # Q^T and K^T tiles: out (128 qdim, 512 tok)
                    for w_sb, raw_dst in ((wq_sb, qt), (wk_sb, kt)):
                        for m in range(2):
                            pq = ps_qk.tile([128, 512], F32)
                            for k in range(NKT):
                                nc.tensor.matmul(
                                    pq[:], w_sb[:, k, 128 * m:128 * (m + 1)], xts[k][:],
                                    start=(k == 0), stop=(k == NKT - 1))
                            raw = qkraw.tile([128, 512], BF16)
                            nc.vector.tensor_copy(raw[:], pq[:])
                            # rope: dst = raw*cs + swap(raw)*ss
                            # swap partner lives 32 partitions away; move it
                            # with 2 strided DMAs off the Activation queue.
                            sw = ropetmp.tile([128, 512], BF16, tag="sw")
                            for blk in range(2):
                                nc.scalar.dma_start(out=sw[64 * blk:64 * blk + 32, :],
                                                    in_=raw[64 * blk + 32:64 * blk + 64, :])
                                nc.scalar.dma_start(out=sw[64 * blk + 32:64 * blk + 64, :],
                                                    in_=raw[64 * blk:64 * blk + 32, :])
                            t1 = ropetmp.tile([128, 512], BF16, tag="t1")
                            nc.vector.tensor_mul(t1[:], raw[:], cs_sb[:, csl])
                            nc.vector.tensor_mul(sw[:], sw[:], ss_sb[:, csl])
                            nc.vector.tensor_add(raw_dst[m][:, csl], t1[:], sw[:])
                    # V tiles: out (128 tok, 256 dims) scattered into 65-stride layout
                    for s_ in range(4):
                        vt = 4 * j + s_
                        pv = ps_v.tile([128, GF], F32)
                        for k in range(NKT):
                            nc.tensor.matmul(
                                pv[:], xts[k][:, 128 * s_:128 * (s_ + 1)], wv_sb[:, k, :],
                                start=(k == 0), stop=(k == NKT - 1))
                        dst = v_sb[:, vt, :].rearrange("p (h e) -> p h e", h=4)[:, :, 0:64]
                        nc.vector.tensor_copy(dst, pv[:].rearrange("p (h e) -> p h e", h=4))

            if debug:
                for m in range(2):
                    nc.sync.dma_start(out=dbg[f"dbg_qt{m}"][:], in_=qt[m][:])
                    nc.sync.dma_start(out=dbg[f"dbg_kt{m}"][:], in_=kt[m][:])
                nc.sync.dma_start(out=dbg["dbg_v"][:],
                                  in_=v_sb.rearrange("p t e -> p (t e)"))

            # ---- phase 2: attention per (q-block qj of 512, pair p) ----
            inv_sqrt_dk = float(1.0 / np.sqrt(DK))
            # one collective per (q-block, head-pair): posted as soon as that
            # pair's context is done, half the payload each. The gathered
            # feature order is [all ranks' p0 | all ranks' p1]; the host
            # permutes Wo rows to match.
            ag_in = dram.tile([NQB, 2, 128, 512], BF16)
            ag_out = dram.tile([NQB, 2, 512, 512], BF16)
            with (
                tc.tile_pool(name="probs", bufs=4) as probspool,
                tc.tile_pool(name="recips", bufs=2) as recips,
                tc.tile_pool(name="ctxu", bufs=3) as ctxupool,
                tc.tile_pool(name="agsb", bufs=2) as agsb,
                tc.tile_pool(name="outsb", bufs=2) as outsb,
                tc.tile_pool(name="ps_sc", bufs=4, space="PSUM") as ps_sc,
                tc.tile_pool(name="ps_ctx", bufs=2, space="PSUM") as ps_ctx,
                tc.tile_pool(name="ps_bc", bufs=1, space="PSUM") as ps_bc,
                tc.tile_pool(name="ps_o", bufs=1, space="PSUM") as ps_o,
            ):
                ag_tiles = {}

                def _outproj(qjo):
                    ag_sb = ag_tiles.pop(qjo)
                    for m in range(2):
                        po = ps_o.tile([128, 512], F32, tag="po", name="po")
                        for k in range(NKT):
                            nc.tensor.matmul(
                                po[:], wo_sb[:, k, 128 * m:128 * (m + 1)], ag_sb[:, k, :],
                                start=(k == 0), stop=(k == NKT - 1))
                        ot = outsb.tile([128, 512], F32, tag="ot", name="ot")
                        nc.scalar.copy(out=ot[:], in_=po[:])
                        nc.sync.dma_start(
                            out=out_d[128 * m:128 * (m + 1), 512 * qjo:512 * (qjo + 1)],
                            in_=ot[:])

                # Schraudolph exp on DVE for head-1 of off-diagonal chunks:
                # bf16(2^e) bit pattern == int16(round(x*log2(e)*128 + 127*128
                # + c)); the convert rides a single tensor_scalar mult+add.
                sch_a = float(np.log2(np.e) * 128.0 * inv_sqrt_dk)
                sch_b = float(127 * 128 - 7.33)
                for qj in range(NQB):
                    qsl = slice(512 * qj, 512 * (qj + 1))
                    # diagonal chunks first (i=0 covers the full 512 q so the
                    # start=True matmul initializes every PSUM element), then
                    # the fully-causal chunks.
                    ch_list = [(4 * qj + i, 128 * i) for i in range(4)] + \
                              [(c, 0) for c in range(4 * qj)]
                    nch = len(ch_list)
                    for p in range(2):
                        pctx = [ps_ctx.tile([65, 512], F32, tag="ctx", name="pctx")
                                for _ in range(2)]
                        for chi, (ch, w0) in enumerate(ch_list):
                            diag = bool(w0) or ch == 4 * qj
                            pscs, prbs = [], []
                            for hh in range(2):
                                psc = ps_sc.tile([128, 512], F32)
                                rsl = slice(64 * hh, 64 * (hh + 1))
                                nc.tensor.matmul(
                                    psc[:, w0:512],
                                    kt[p][rsl, 128 * ch:128 * (ch + 1)],
                                    qt[p][rsl, 512 * qj + w0:512 * (qj + 1)],
                                    start=True, stop=True)
                                pscs.append(psc)
                            for hh in range(2):
                                if hh == 1 and not diag:
                                    pri = probspool.tile([128, 512], mybir.dt.int16,
                                                         tag="pri")
                                    nc.vector.tensor_scalar(
                                        out=pri[:, w0:512], in0=pscs[hh][:, w0:512],
                                        scalar1=sch_a, scalar2=sch_b,
                                        op0=mybir.AluOpType.mult,
                                        op1=mybir.AluOpType.add)
                                    probs = pri[:].bitcast(BF16)
                                else:
                                    probs = probspool.tile([128, 512], BF16, tag="prb")
                                    nc.scalar.activation(out=probs[:, w0:512],
                                                         in_=pscs[hh][:, w0:512],
                                                         func=AF.Exp, scale=inv_sqrt_dk)
                                    if diag:
                                        sl = probs[:, w0:w0 + 128]
                                        nc.vector.tensor_mul(sl, sl, tri_sb[:])
                                prbs.append(probs)
                            for hh in range(2):
                                h65 = 65 * (2 * p + hh)
                                nc.tensor.matmul(
                                    pctx[hh][:, w0:512],
                                    v_sb[:, ch, h65:h65 + 65],
                                    prbs[hh][:, w0:512],
                                    start=(chi == 0), stop=(chi == nch - 1))
                        # normalize: denominator rode the PV matmul as row 64
                        recip = recips.tile([128, 1024], F32, tag="recip", name="recip")
                        recipr = recips.tile([128, 1024], BF16, tag="recipr", name="recipr")
                        ctxus = []
                        for hh in range(2):
                            ctxu = ctxupool.tile([65, 512], F32, tag="ctxu", name="ctxu")
                            nc.scalar.copy(out=ctxu[:], in_=pctx[hh][:])
                            nc.vector.reciprocal_approx_fast(
                                out=recip[0:65, 512 * hh:512 * (hh + 1)],
                                in_=ctxu[:])
                            ctxus.append(ctxu)
                        nc.vector.tensor_copy(recipr[64:65, :], recip[64:65, :])
                        if debug:
                            nc.sync.dma_start(out=dbg["dbg_rcp"][2 * qj + p, None, :],
                                              in_=recipr[64:65, :])
                            for hh in range(2):
                                nc.sync.dma_start(
                                    out=dbg["dbg_den"][2 * qj + p, None,
                                                       512 * hh:512 * (hh + 1)],
                                    in_=ctxus[hh][64:65, :])
                        for hh in range(2):
                            pbc = ps_bc.tile([64, 512], F32, tag="bc")
                            nc.tensor.matmul(
                                pbc[:], ones_sb[64:65, 0:64],
                                recipr[64:65, 512 * hh:512 * (hh + 1)],
                                start=True, stop=True)
                            if debug:
                                pbs = recips.tile([128, 512], F32, tag="pbs", name="pbs")
                                nc.vector.tensor_copy(pbs[0:1, :], pbc[0:1, :])
                                nc.sync.dma_start(
                                    out=dbg["dbg_pbc"][2 * qj + p, None,
                                                       512 * hh:512 * (hh + 1)],
                                    in_=pbs[0:1, :])
                            nc.vector.tensor_mul(
                                ctx2_sb[p][64 * hh:64 * (hh + 1), qsl],
                                ctxus[hh][0:64, :], pbc[:])
                        # ship this pair's context slab and gather it now;
                        # the SBUF reload is issued immediately (gpsimd
                        # queue) so it completes right after the AG does.
                        nc.sync.dma_start(out=ag_in[qj, p], in_=ctx2_sb[p][:, qsl])
                        nc.gpsimd.collective_compute(
                            "AllGather", mybir.AluOpType.bypass,
                            replica_groups=[[0, 1, 2, 3], [4, 5, 6, 7]],
                            ins=[ag_in[qj, p].opt()], outs=[ag_out[qj, p].opt()])
                        if p == 0:
                            ag_tiles[qj] = agsb.tile([128, NKT, 512], BF16,
                                                     tag="ag", name="ag")
                        nc.gpsimd.dma_start(
                            out=ag_tiles[qj][:, 4 * p:4 * (p + 1), :],
                            in_=ag_out[qj, p].rearrange("(k p) n -> p k n", p=128))
                    # output projection pipelined one q-block behind the AG.
                    if qj >= 1:
                        _outproj(qj - 1)
                    if qj == NQB - 1:
                        _outproj(qj)
                if debug:
                    for p in range(2):
                        nc.sync.dma_start(out=dbg[f"dbg_ctx{p}"][:], in_=ctx2_sb[p][:])


def _build(debug: bool = False):
    nc = bacc.Bacc("TRN2", target_bir_lowering=False, debug=False, num_devices=NCORES)
    _emit(nc, debug=debug)
    nc.compile()
    return nc


def _perm_rows(g: int) -> np.ndarray:
    rows = []
    for l in range(HPC):
        h = HPC * g + l
        rows += [DK * h + d for d in range(0, DK, 2)]
        rows += [DK * h + d for d in range(1, DK, 2)]
    return np.asarray(rows)


def _wo_perm() -> np.ndarray:
    # gathered ctx feature order: [p0: rank0(h 0,1) rank1(h 4,5) ... | p1: ...]
    perm = []
    for p in range(2):
        for r in range(4):
            for hh in range(2):
                h = 4 * r + 2 * p + hh
                perm += [DK * h + d for d in range(DK)]
    return np.asarray(perm)


def kernel(x, token_positions, Wq, Wk, Wv, Wo):
    bf = mybir.dt.np(BF16)
    x = np.asarray(x, dtype=np.float32)
    Wq = np.asarray(Wq, dtype=np.float32)
    Wk = np.asarray(Wk, dtype=np.float32)
    Wv = np.asarray(Wv, dtype=np.float32)
    Wo = np.asarray(Wo, dtype=np.float32)
    pos = np.asarray(token_positions).astype(np.float64)

    debug = os.environ.get("KERNEL_DEBUG", "0") == "1"
    if "nc" not in _CACHE:
        _CACHE["nc"] = _build(debug=debug)
    nc = _CACHE["nc"]

    inv_freq = np.exp(np.arange(0, DK, 2, dtype=np.float32) * (-np.log(THETA) / DK)).astype(np.float64)
    ang = pos[:, None] * inv_freq[None, :]              # (S, 32)
    cos_t = np.cos(ang).astype(np.float32).T            # (32, S)
    sin_t = np.sin(ang).astype(np.float32).T
    fi = np.arange(128) % NF
    half = (np.arange(128) // NF) % 2
    CS = np.ascontiguousarray(cos_t[fi, :]).astype(bf)
    SS = np.ascontiguousarray(
        np.where(half[:, None] == 0, -sin_t[fi, :], sin_t[fi, :])).astype(bf)
    ONES = np.ones((128, 64), dtype=np.float32).astype(bf)
    TRI = np.triu(np.ones((128, 128), dtype=np.float32)).astype(bf)  # keep k<=q

    in_maps = []
    for c in range(NCORES):
        b, g = divmod(c, 4)
        pr = _perm_rows(g)
        in_maps.append({
            "xT": np.ascontiguousarray(x[b].T).astype(bf),
            "wqT": np.ascontiguousarray(Wq[pr].T).astype(bf),
            "wkT": np.ascontiguousarray(Wk[pr].T).astype(bf),
            "wvT": np.ascontiguousarray(Wv[GF * g:GF * (g + 1)].T).astype(bf),
            "woT": np.ascontiguousarray(Wo[GF * g:GF * (g + 1)][:, _wo_perm()].T).astype(bf),
            "cs": CS, "ss": SS, "ones": ONES, "tri": TRI,
        })

    trace = os.environ.get("KERNEL_TRACE", "0") == "1"
    res = run_bass_kernel_spmd(nc, in_maps, list(range(NCORES)), trace=trace)
    _CACHE["last_result"] = res

    out = np.empty((B, S, D), dtype=np.float32)
    for c in range(NCORES):
        b, g = divmod(c, 4)
        out[b, :, GF * g:GF * (g + 1)] = res.results[c]["out"].T
    return out
